# revision 1
# baseline (speedup 1.0000x reference)
"""Trainium2 Bass kernel for EnhancedMetaWeightNetwork.

Full (unsharded) inputs in, full output out. Internally: 8 NeuronCores,
core c handles batch b = c // 2 and query-row half c % 2 (1024 rows).
Attention K/V are computed per-core for the full sequence of the core's
batch (duplicated across the 2 cores sharing a batch; no collectives).

Layout strategy on each core (fp32 storage, fp32r matmuls):
  - activations kept feature-major ("T" = [feature, token]) for the
    attention/projection chain, token-major for the meta-MLP layernorms.
  - scoresT [key, query] per head; softmax denominator via ones-matmul
    restream on the PE; reciprocal broadcast via rank-1 ones matmul.
  - importance lookup via indirect DMA gather from the vocab table.
  - K^T, Q^T, ctx^T spilled through DRAM scratch to fit SBUF.
"""

import numpy as np

H = 1024
NH = 8
HD = 128           # head dim
S = 2048           # keys / full sequence
SQ = 1024          # own query rows per core
MD = 256           # meta dim
MD2 = 128
VOCAB = 32000
MIN_W, MAX_W = 0.1, 5.0
LN_EPS = 1e-5
P = 128
INV_SQRT_HD = 1.0 / np.sqrt(np.float32(HD))

_CACHE = {}


def _build(stop=None):
    """stop in {None, "x", "qkv", "att", "o"}: truncate after that phase
    (debug bisection; a dummy zero output is written instead)."""
    import concourse.bass as bass
    import concourse.mybir as mybir
    import concourse.tile as tile
    from concourse import bacc
    from concourse.masks import make_identity

    f32 = mybir.dt.float32
    f32r = mybir.dt.float32r
    i32 = mybir.dt.int32
    OP = mybir.AluOpType
    ACT = mybir.ActivationFunctionType

    order = {"x": 0, "qkv": 1, "att": 2, "o": 3, "m1": 4, "m2": 5, None: 9}
    lvl = order[stop]

    nc = bacc.Bacc("TRN2", target_bir_lowering=False, debug=False,
                   enable_asserts=False, num_devices=8)

    # ---------------- DRAM parameters ----------------
    dp = nc.declare_dram_parameter
    hT = dp("hT", [H, S], f32, isOutput=False)            # hidden[b].T (own half first)
    pT = dp("pT", [H, S], f32, isOutput=False)            # pos_embed[0].T (own half first)
    wqT = dp("wqT", [H, H], f32r, isOutput=False)         # in_proj_w[0:H].T
    wkT = dp("wkT", [H, H], f32r, isOutput=False)
    wvT = dp("wvT", [H, H], f32r, isOutput=False)
    bq_c = dp("bq_c", [P, H // P], f32, isOutput=False)   # bias, partition-major
    bk_c = dp("bk_c", [P, H // P], f32, isOutput=False)
    bv_b = dp("bv_b", [P, H], f32, isOutput=False)        # bias bcast over partitions
    owT = dp("owT", [H, H], f32r, isOutput=False)         # out_w.T
    ob_c = dp("ob_c", [P, H // P], f32, isOutput=False)
    w1T = dp("w1T", [2 * H, MD], f32r, isOutput=False)
    b1_cd = dp("b1_cd", [P, MD // P], f32, isOutput=False)
    g1_cd = dp("g1_cd", [P, MD // P], f32, isOutput=False)
    be1_cd = dp("be1_cd", [P, MD // P], f32, isOutput=False)
    w2T = dp("w2T", [MD, MD2], f32r, isOutput=False)
    b2_b = dp("b2_b", [P, MD2], f32, isOutput=False)
    g2_b = dp("g2_b", [P, MD2], f32, isOutput=False)
    be2_b = dp("be2_b", [P, MD2], f32, isOutput=False)
    w3_b = dp("w3_b", [P, MD2], f32, isOutput=False)
    b3_c = dp("b3_c", [P, 1], f32, isOutput=False)
    kbias = dp("kbias", [P, S // P], f32, isOutput=False)  # attn mask bias per key
    maskf = dp("maskf", [P, SQ // P], f32, isOutput=False)
    tok = dp("tok", [SQ, 1], i32, isOutput=False)
    table = dp("table", [VOCAB, 1], f32, isOutput=False)
    out = dp("out", [SQ], f32, isOutput=True)

    NKT = S // P          # 16 key tiles
    NC8 = H // P          # 8 feature chunks
    NTT = SQ // P         # 8 own token tiles

    # DRAM scratch
    ktd = nc.dram_tensor("ktd", [H, S], f32r)      # K^T spill
    qtd = nc.dram_tensor("qtd", [H, SQ], f32r)     # Q^T spill
    ctd = nc.dram_tensor("ctd", [H, SQ], f32r)     # ctx^T spill
    atd = nc.dram_tensor("atd", [H, SQ], f32r)     # attended^T spill

    with tile.TileContext(nc) as tc:
        with tc.tile_pool(name="const", bufs=1) as cst, \
             tc.tile_pool(name="xown", bufs=1) as xop:

            # ---------------- constants ----------------
            ones_f = cst.tile([P, P], f32, tag="ones_f")
            nc.any.memset(ones_f[:], 1.0)
            ones_r = cst.tile([P, P], f32r, tag="ones_r")
            nc.vector.tensor_copy(ones_r[:], ones_f[:])
            ident = cst.tile([P, P], f32, tag="ident")
            make_identity(nc, ident[:])
            eps_sb = cst.tile([P, 1], f32, tag="eps")
            nc.any.memset(eps_sb[:], LN_EPS)

            def cload(shape, tag, src):
                t = cst.tile(shape, f32, tag=tag)
                nc.sync.dma_start(t[:], src[:])
                return t

            kbias_sb = cload([P, NKT], "kbias", kbias)
            maskf_sb = cload([P, NTT], "maskf", maskf)
            b3_sb = cload([P, 1], "b3", b3_c)
            w3_sb = cload([P, MD2], "w3", w3_b)
            bq_sb = cload([P, NC8], "bq", bq_c)
            bk_sb = cload([P, NC8], "bk", bk_c)
            ob_sb = cload([P, NC8], "ob", ob_c)
            bv_sb = cload([P, H], "bv", bv_b)
            b1_c = cload([P, MD // P], "b1c", b1_cd)
            g1_c = cload([P, MD // P], "g1c", g1_cd)
            be1_c = cload([P, MD // P], "be1c", be1_cd)
            b2_sb = cload([P, MD2], "b2", b2_b)
            g2_sb = cload([P, MD2], "g2", g2_b)
            be2_sb = cload([P, MD2], "be2", be2_b)

            imp_all = cst.tile([P, NTT], f32, tag="imp_all")
            if lvl >= 9:
                for tt in range(NTT):
                    itt = cst.tile([P, 1], i32, tag=f"it{tt}")
                    nc.sync.dma_start(itt[:], tok[tt * P:(tt + 1) * P, :])
                    nc.gpsimd.indirect_dma_start(
                        out=imp_all[:, tt:tt + 1], out_offset=None, in_=table[:],
                        in_offset=bass.IndirectOffsetOnAxis(ap=itt[:, :1], axis=0))

            if lvl < 5:
                dout = cst.tile([P, NTT], f32, tag="dout")
                nc.any.memset(dout[:], 0.0)
                nc.sync.dma_start(out[:].rearrange("(t p) -> p t", p=P), dout[:])

            x_own = xop.tile([P, NC8, SQ], f32r, tag="x_own")

            with tc.tile_pool(name="vpool", bufs=1) as vp:
                v_sb = vp.tile([P, NKT, H], f32r, tag="v")

                with tc.tile_pool(name="xoth", bufs=1) as xot, \
                     tc.tile_pool(name="ps_mm1", bufs=6, space="PSUM") as ps1:
                    x_oth = xot.tile([P, NC8, S - SQ], f32r, tag="x_oth")

                    # ---------- phase X ----------
                    with tc.tile_pool(name="tmpx", bufs=3) as tmp:
                        for c8 in range(NC8):
                            for half, xdst in ((0, x_own), (1, x_oth)):
                                ht = tmp.tile([P, SQ], f32, tag="ht")
                                pt = tmp.tile([P, SQ], f32, tag="pt")
                                cs = half * SQ
                                nc.sync.dma_start(ht[:], hT[c8 * P:(c8 + 1) * P, cs:cs + SQ])
                                nc.sync.dma_start(pt[:], pT[c8 * P:(c8 + 1) * P, cs:cs + SQ])
                                nc.vector.tensor_tensor(out=xdst[:, c8, :], in0=ht[:],
                                                        in1=pt[:], op=OP.add)

                    # ---------- phases Q/K/V ----------
                    with tc.tile_pool(name="wqkv", bufs=2) as wst, \
                         tc.tile_pool(name="wvp", bufs=1) as wvp, \
                         tc.tile_pool(name="stgqk", bufs=4) as stg:
                        if lvl >= 1:
                            wv_sb = wvp.tile([P, NC8, H], f32r, tag="wv")
                            for db in range(H // 512):
                                nc.sync.dma_start(
                                    wv_sb[:, :, db * 512:(db + 1) * 512],
                                    wvT[:, db * 512:(db + 1) * 512]
                                    .rearrange("(c p) n -> p c n", p=P))
                        for dt in range(NC8 if lvl >= 1 else 0):
                            wq_sb = wst.tile([P, NC8, P], f32r, tag="wq")
                            nc.sync.dma_start(wq_sb[:], wqT[:, dt * P:(dt + 1) * P]
                                              .rearrange("(c p) n -> p c n", p=P))
                            psqs = [ps1.tile([P, 512], mybir.dt.float32, tag="mm512",
                                              name=f"psq{qb}") for qb in range(SQ // 512)]
                            for c8 in range(NC8):
                                for qb in range(SQ // 512):
                                    nc.tensor.matmul(psqs[qb][:],
                                                     lhsT=wq_sb[:, c8, :],
                                                     rhs=x_own[:, c8, qb * 512:(qb + 1) * 512],
                                                     start=(c8 == 0), stop=(c8 == NC8 - 1))
                            for qb in range(SQ // 512):
                                qstg = stg.tile([P, 512], f32r, tag="qstg")
                                nc.scalar.activation(qstg[:], psqs[qb][:], ACT.Identity,
                                                     bias=bq_sb[:, dt:dt + 1],
                                                     scale=INV_SQRT_HD)
                                nc.sync.dma_start(
                                    qtd[dt * P:(dt + 1) * P, qb * 512:(qb + 1) * 512],
                                    qstg[:])

                    # ---------- phase K ----------
                        for dt in range(NC8 if lvl >= 1 else 0):
                            wk_sb = wst.tile([P, NC8, P], f32r, tag="wk")
                            nc.sync.dma_start(wk_sb[:], wkT[:, dt * P:(dt + 1) * P]
                                              .rearrange("(c p) n -> p c n", p=P))
                            psks = [ps1.tile([P, 512], mybir.dt.float32, tag="mm512",
                                              name=f"psk{sb}") for sb in range(S // 512)]
                            for c8 in range(NC8):
                                for sb in range(S // 512):
                                    if sb < SQ // 512:
                                        rhs = x_own[:, c8, sb * 512:(sb + 1) * 512]
                                    else:
                                        rhs = x_oth[:, c8, (sb - SQ // 512) * 512:
                                                    (sb - SQ // 512 + 1) * 512]
                                    nc.tensor.matmul(psks[sb][:], lhsT=wk_sb[:, c8, :],
                                                     rhs=rhs,
                                                     start=(c8 == 0), stop=(c8 == NC8 - 1))
                            for sb in range(S // 512):
                                kstg = stg.tile([P, 512], f32r, tag="kstg")
                                nc.scalar.activation(kstg[:], psks[sb][:], ACT.Identity,
                                                     bias=bk_sb[:, dt:dt + 1], scale=1.0)
                                nc.sync.dma_start(
                                    ktd[dt * P:(dt + 1) * P, sb * 512:(sb + 1) * 512],
                                    kstg[:])

                    # ---------- phase V ----------
                        for tt in range(NKT if lvl >= 1 else 0):
                            psvs = [ps1.tile([P, 512], mybir.dt.float32, tag="mm512",
                                             name=f"psv{db}") for db in range(H // 512)]
                            for c8 in range(NC8):
                                if tt < NTT:
                                    lhsT = x_own[:, c8, tt * P:(tt + 1) * P]
                                else:
                                    lhsT = x_oth[:, c8, (tt - NTT) * P:(tt - NTT + 1) * P]
                                for db in range(H // 512):
                                    nc.tensor.matmul(psvs[db][:], lhsT=lhsT,
                                                     rhs=wv_sb[:, c8, db * 512:(db + 1) * 512],
                                                     start=(c8 == 0), stop=(c8 == NC8 - 1))
                            for db in range(H // 512):
                                nc.vector.tensor_tensor(
                                    out=v_sb[:, tt, db * 512:(db + 1) * 512],
                                    in0=psvs[db][:],
                                    in1=bv_sb[:, db * 512:(db + 1) * 512],
                                    op=OP.add)

                # ---------- attention (x_oth freed); ow/ctx prefetch ----------
                with tc.tile_pool(name="owp", bufs=1) as owp, \
                     tc.tile_pool(name="ctxq0", bufs=1) as cxq0:
                    if lvl >= 3:
                        ow_sb = owp.tile([P, NC8, H], f32r, tag="ow")
                        ctx_qb0 = cxq0.tile([P, NC8, 512], f32r, tag="ctx_qb0")
                    with tc.tile_pool(name="kqs", bufs=2) as kqs, \
                         tc.tile_pool(name="exps", bufs=6) as exps, \
                         tc.tile_pool(name="asml", bufs=2) as asml, \
                         tc.tile_pool(name="ps_sc", bufs=5, space="PSUM") as ps_sc, \
                         tc.tile_pool(name="ps_ctx", bufs=2, space="PSUM") as ps_ctx, \
                         tc.tile_pool(name="ps_dn", bufs=1, space="PSUM") as ps_dn:
                        for h in range(NH if lvl >= 2 else 0):
                            kt_h = kqs.tile([P, S], f32r, tag="kt_h")
                            nc.sync.dma_start(kt_h[:], ktd[h * P:(h + 1) * P, :])
                            qt_h = kqs.tile([P, SQ], f32r, tag="qt_h")
                            nc.sync.dma_start(qt_h[:], qtd[h * P:(h + 1) * P, :])

                            for qb in range(SQ // 512):
                                qsl = slice(qb * 512, (qb + 1) * 512)
                                cps = ps_ctx.tile([P, 512], mybir.dt.float32, tag="cps")
                                dn = ps_dn.tile([P, 512], mybir.dt.float32, tag="dn")
                                for kt in range(NKT):
                                    sc = ps_sc.tile([P, 512], mybir.dt.float32, tag="sc")
                                    nc.tensor.matmul(sc[:],
                                                     lhsT=kt_h[:, kt * P:(kt + 1) * P],
                                                     rhs=qt_h[:, qsl],
                                                     start=True, stop=True)
                                    ex = exps.tile([P, 512], f32r, tag="ex")
                                    nc.scalar.activation(ex[:], sc[:], ACT.Exp,
                                                         bias=kbias_sb[:, kt:kt + 1],
                                                         scale=1.0)
                                    nc.tensor.matmul(cps[:],
                                                     lhsT=v_sb[:, kt, h * P:(h + 1) * P],
                                                     rhs=ex[:],
                                                     start=(kt == 0), stop=(kt == NKT - 1))
                                    nc.tensor.matmul(dn[:],
                                                     lhsT=ones_r[:],
                                                     rhs=ex[:],
                                                     start=(kt == 0), stop=(kt == NKT - 1))
                                cpsc = asml.tile([P, 512], f32, tag="cpsc")
                                nc.vector.tensor_copy(cpsc[:], cps[:])
                                dnc = asml.tile([P, 512], f32, tag="dnc")
                                nc.vector.tensor_copy(dnc[:], dn[:])
                                rcb = asml.tile([P, 512], f32, tag="rcb")
                                with nc.allow_low_precision(reason="fp32 storage"):
                                    nc.vector.reciprocal(rcb[:], dnc[:])
                                cstg = asml.tile([P, 512], f32r, tag="cstg")
                                nc.vector.tensor_tensor(out=cstg[:], in0=cpsc[:],
                                                        in1=rcb[:], op=OP.mult)
                                nc.sync.dma_start(ctd[h * P:(h + 1) * P, qsl], cstg[:])
                                if lvl >= 3 and qb == 0:
                                    nc.sync.dma_start(
                                        ow_sb[:, :, h * P:(h + 1) * P],
                                        owT[:, h * P:(h + 1) * P]
                                        .rearrange("(c p) n -> p c n", p=P))
                                    nc.sync.dma_start(ctx_qb0[:, h, :],
                                                      ctd[h * P:(h + 1) * P, 0:512])

                    # ---------- out-projection -> atd (DRAM) ----------
                    with tc.tile_pool(name="ctxq", bufs=1) as cxq, \
                         tc.tile_pool(name="ostg", bufs=4) as ostg, \
                         tc.tile_pool(name="ps_o", bufs=6, space="PSUM") as ps_o:
                        if lvl >= 3:
                            ctx_qb1 = cxq.tile([P, NC8, 512], f32r, tag="ctx_qb1")
                            for c8 in range(NC8):
                                nc.sync.dma_start(ctx_qb1[:, c8, :],
                                                  ctd[c8 * P:(c8 + 1) * P, 512:1024])
                            ctx_qbs = [ctx_qb0, ctx_qb1]
                            for qb in range(SQ // 512):
                                for dt in range(NC8):
                                    pso = ps_o.tile([P, 512], mybir.dt.float32,
                                                    tag="mm512", name="pso")
                                    for c8 in range(NC8):
                                        nc.tensor.matmul(
                                            pso[:],
                                            lhsT=ow_sb[:, c8, dt * P:(dt + 1) * P],
                                            rhs=ctx_qbs[qb][:, c8, :],
                                            start=(c8 == 0), stop=(c8 == NC8 - 1))
                                    osg = ostg.tile([P, 512], f32r, tag="osg")
                                    nc.scalar.activation(osg[:], pso[:],
                                                         ACT.Identity,
                                                         bias=ob_sb[:, dt:dt + 1],
                                                         scale=1.0)
                                    nc.sync.dma_start(
                                        atd[dt * P:(dt + 1) * P,
                                            qb * 512:(qb + 1) * 512], osg[:])

            # ---------- meta MLP ----------
            with tc.tile_pool(name="mw", bufs=1) as mw, \
                 tc.tile_pool(name="msml", bufs=3) as sml, \
                 tc.tile_pool(name="attq", bufs=1) as atq, \
                 tc.tile_pool(name="ps_m", bufs=6, space="PSUM") as ps2:
                if lvl >= 4:
                    w1_sb = mw.tile([P, 2 * NC8, MD], f32r, tag="w1")
                    nc.sync.dma_start(w1_sb[:],
                                      w1T[:].rearrange("(c p) n -> p c n", p=P))
                    w2_sb = mw.tile([P, MD // P, MD2], f32r, tag="w2")
                    nc.sync.dma_start(w2_sb[:],
                                      w2T[:].rearrange("(c p) n -> p c n", p=P))
                    res_sb = mw.tile([P, NTT], f32, tag="res")

                    # ---- h1 in feature-major: h1preT [256, SQ] ----
                    att_qbs = []
                    for qb in range(SQ // 512):
                        att_qb = atq.tile([P, NC8, 512], f32r, tag=f"att_qb{qb}")
                        for c8 in range(NC8):
                            nc.sync.dma_start(
                                att_qb[:, c8, :],
                                atd[c8 * P:(c8 + 1) * P, qb * 512:(qb + 1) * 512])
                        att_qbs.append(att_qb)
                    NFT = MD // P      # 2 feature tiles of h1
                    h1p = mw.tile([P, NFT, SQ], f32r, tag="h1p")
                    h1sq = mw.tile([P, NFT, SQ], f32r, tag="h1x")
                    h1n = mw.tile([P, NFT, SQ], f32r, tag="h1x", name="h1n")
                    stat = mw.tile([P, 3, SQ], f32, tag="stat")
                    nmean, work, m2r = stat[:, 0, :], stat[:, 1, :], stat[:, 2, :]
                    ex2m = varm = rstd = work

                    for ft in range(NFT):
                        for qb in range(SQ // 512):
                            psf_t = ps2.tile([P, 512], mybir.dt.float32,
                                             tag="mm512", name="psf")
                            for c16 in range(2 * NC8):
                                if c16 < NC8:
                                    rhs = x_own[:, c16, qb * 512:(qb + 1) * 512]
                                else:
                                    rhs = att_qbs[qb][:, c16 - NC8, :]
                                nc.tensor.matmul(
                                    psf_t[:],
                                    lhsT=w1_sb[:, c16, ft * P:(ft + 1) * P],
                                    rhs=rhs,
                                    start=(c16 == 0), stop=(c16 == 2 * NC8 - 1))
                            nc.scalar.activation(
                                h1p[:, ft, qb * 512:(qb + 1) * 512], psf_t[:],
                                ACT.Identity, bias=b1_c[:, ft:ft + 1], scale=1.0)
                    for qb in range(SQ // 512):
                        qsl = slice(qb * 512, (qb + 1) * 512)
                        for ft in range(NFT):
                            nc.vector.tensor_tensor(out=h1sq[:, ft, qsl],
                                                    in0=h1p[:, ft, qsl],
                                                    in1=h1p[:, ft, qsl], op=OP.mult)
                        psA = ps2.tile([P, 512], mybir.dt.float32, tag="mm512",
                                       name="psA")
                        psB = ps2.tile([P, 512], mybir.dt.float32, tag="mm512",
                                       name="psB")
                        for ft in range(NFT):
                            nc.tensor.matmul(psA[:], lhsT=ones_r[:],
                                             rhs=h1p[:, ft, qsl],
                                             start=(ft == 0), stop=(ft == NFT - 1))
                        for ft in range(NFT):
                            nc.tensor.matmul(psB[:], lhsT=ones_r[:],
                                             rhs=h1sq[:, ft, qsl],
                                             start=(ft == 0), stop=(ft == NFT - 1))
                        nc.vector.tensor_scalar_mul(nmean[:, qsl], psA[:],
                                                    -1.0 / MD)
                        nc.vector.tensor_scalar_mul(ex2m[:, qsl], psB[:], 1.0 / MD)
                        nc.vector.tensor_tensor(out=m2r[:, qsl], in0=nmean[:, qsl],
                                                in1=nmean[:, qsl], op=OP.mult)
                        nc.vector.tensor_tensor(out=work[:, qsl], in0=work[:, qsl],
                                                in1=m2r[:, qsl], op=OP.subtract)
                        # rstd = exp(-0.5 * ln(var + eps)) on ACT (fast path)
                        nc.scalar.activation(varm[:, qsl], varm[:, qsl], ACT.Ln,
                                             bias=eps_sb[:, 0:1], scale=1.0)
                        nc.scalar.activation(rstd[:, qsl], varm[:, qsl], ACT.Exp,
                                             bias=0.0, scale=-0.5)
                        for ft in range(NFT):
                            nc.vector.tensor_tensor(out=h1n[:, ft, qsl],
                                                    in0=h1p[:, ft, qsl],
                                                    in1=nmean[:, qsl], op=OP.add)
                            nc.vector.tensor_tensor(out=h1n[:, ft, qsl],
                                                    in0=h1n[:, ft, qsl],
                                                    in1=rstd[:, qsl], op=OP.mult)
                            nc.scalar.activation(h1n[:, ft, qsl], h1n[:, ft, qsl],
                                                 ACT.Relu, bias=be1_c[:, ft:ft + 1],
                                                 scale=g1_c[:, ft:ft + 1])

                # ---- h2 + batched LN2/final across all tiles ----
                hb2_all = mw.tile([P, NTT, MD2], f32, tag="hb2_all")
                for tt in range(NTT if lvl >= 5 else 0):
                    ph2_t = ps2.tile([P, 512], mybir.dt.float32, tag="mm512",
                                     name="ph2")
                    ph2 = ph2_t[:, :MD2]
                    for ft in range(MD // P):
                        nc.tensor.matmul(ph2, lhsT=h1n[:, ft, tt * P:(tt + 1) * P],
                                         rhs=w2_sb[:, ft, :],
                                         start=(ft == 0), stop=(ft == MD // P - 1))
                    nc.vector.scalar_tensor_tensor(out=hb2_all[:, tt, :], in0=ph2,
                                                   scalar=1.0, in1=b2_sb[:],
                                                   op0=OP.mult, op1=OP.add)
                if lvl >= 5:
                    F2 = float(MD2)
                    sums2 = sml.tile([P, NTT], f32, tag="sums2")
                    nc.vector.reduce_sum(sums2[:], hb2_all[:],
                                         axis=mybir.AxisListType.X)
                    msq = sml.tile([P, NTT, MD2], f32, tag="msq")
                    ssq2 = sml.tile([P, NTT], f32, tag="ssq2")
                    nc.vector.tensor_tensor(out=msq[:], in0=hb2_all[:],
                                            in1=hb2_all[:], op=OP.mult)
                    nc.vector.reduce_sum(ssq2[:], msq[:], axis=mybir.AxisListType.X)
                    nm2 = sml.tile([P, NTT], f32, tag="nm2")
                    nc.vector.tensor_scalar_mul(nm2[:], sums2[:], -1.0 / F2)
                    ex22 = sml.tile([P, NTT], f32, tag="ex22")
                    nc.vector.tensor_scalar_mul(ex22[:], ssq2[:], 1.0 / F2)
                    mm2 = sml.tile([P, NTT], f32, tag="mm2")
                    nc.vector.tensor_tensor(out=mm2[:], in0=nm2[:], in1=nm2[:],
                                            op=OP.mult)
                    var2 = sml.tile([P, NTT], f32, tag="var2")
                    nc.vector.tensor_tensor(out=var2[:], in0=ex22[:], in1=mm2[:],
                                            op=OP.subtract)
                    std2 = sml.tile([P, NTT], f32, tag="std2")
                    nc.scalar.activation(std2[:], var2[:], ACT.Sqrt,
                                         bias=eps_sb[:, 0:1], scale=1.0)
                    rstd2 = sml.tile([P, NTT], f32, tag="rstd2")
                    nc.vector.reciprocal(rstd2[:], std2[:])
                    t1a = sml.tile([P, NTT, MD2], f32, tag="t1a")
                    nc.vector.tensor_tensor(
                        out=t1a[:], in0=hb2_all[:],
                        in1=nm2[:, :, None].to_broadcast([P, NTT, MD2]),
                        op=OP.add)
                    nc.vector.tensor_tensor(
                        out=t1a[:], in0=t1a[:],
                        in1=rstd2[:, :, None].to_broadcast([P, NTT, MD2]),
                        op=OP.mult)
                    nc.vector.tensor_tensor(
                        out=t1a[:], in0=t1a[:],
                        in1=g2_sb[:, None, :].to_broadcast([P, NTT, MD2]),
                        op=OP.mult)
                    nc.vector.tensor_tensor(
                        out=t1a[:], in0=t1a[:],
                        in1=be2_sb[:, None, :].to_broadcast([P, NTT, MD2]),
                        op=OP.add)
                    nc.vector.tensor_scalar_max(t1a[:], t1a[:], 0.0)
                    nc.vector.tensor_tensor(
                        out=t1a[:], in0=t1a[:],
                        in1=w3_sb[:, None, :].to_broadcast([P, NTT, MD2]),
                        op=OP.mult)
                    base8 = sml.tile([P, NTT], f32, tag="base8")
                    nc.vector.reduce_sum(base8[:], t1a[:], axis=mybir.AxisListType.X)
                    nc.vector.tensor_tensor(
                        out=base8[:], in0=base8[:],
                        in1=b3_sb[:, 0:1].to_broadcast([P, NTT]), op=OP.add)
                    imp1a = sml.tile([P, NTT], f32, tag="imp1a")
                    nc.vector.tensor_scalar_add(imp1a[:], imp_all[:], 1.0)
                    nc.vector.tensor_tensor(out=base8[:], in0=base8[:],
                                            in1=imp1a[:], op=OP.mult)
                    nc.vector.tensor_scalar(base8[:], base8[:], MAX_W, MIN_W,
                                            op0=OP.min, op1=OP.max)
                    nc.vector.tensor_tensor(out=res_sb[:], in0=base8[:],
                                            in1=maskf_sb[:], op=OP.mult)
                    nc.sync.dma_start(out[:].rearrange("(t p) -> p t", p=P),
                                      res_sb[:])

    nc.compile()
    return nc


def _get_program():
    import os
    stop = os.environ.get("KB_STOP") or None
    key = ("nc", stop)
    if key not in _CACHE:
        _CACHE[key] = _build(stop)
    return _CACHE[key]


def _prep_in_maps(inputs):
    hidden = np.ascontiguousarray(np.asarray(inputs["hidden_states"], dtype=np.float32))
    token_ids = np.asarray(inputs["token_ids"], dtype=np.int32)
    mask = np.asarray(inputs["attention_mask"]).astype(bool)
    pos = np.asarray(inputs["pos_embed"], dtype=np.float32)
    in_proj_w = np.asarray(inputs["in_proj_w"], dtype=np.float32)
    in_proj_b = np.asarray(inputs["in_proj_b"], dtype=np.float32)
    out_w = np.asarray(inputs["out_w"], dtype=np.float32)
    out_b = np.asarray(inputs["out_b"], dtype=np.float32)
    w1 = np.asarray(inputs["w1"], dtype=np.float32)
    b1 = np.asarray(inputs["b1"], dtype=np.float32)
    g1 = np.asarray(inputs["g1"], dtype=np.float32)
    beta1 = np.asarray(inputs["beta1"], dtype=np.float32)
    w2 = np.asarray(inputs["w2"], dtype=np.float32)
    b2 = np.asarray(inputs["b2"], dtype=np.float32)
    g2 = np.asarray(inputs["g2"], dtype=np.float32)
    beta2 = np.asarray(inputs["beta2"], dtype=np.float32)
    w3 = np.asarray(inputs["w3"], dtype=np.float32)
    b3 = np.asarray(inputs["b3"], dtype=np.float32)
    table = np.asarray(inputs["importance_table"], dtype=np.float32)

    B, S_, H_ = hidden.shape
    assert (B, S_, H_) == (4, S, H), (B, S_, H_)

    posT = np.ascontiguousarray(pos[0].T)                      # [H, S]
    wqT = np.ascontiguousarray(in_proj_w[0:H].T)               # [H, H]
    wkT = np.ascontiguousarray(in_proj_w[H:2 * H].T)
    wvT = np.ascontiguousarray(in_proj_w[2 * H:3 * H].T)
    bq = in_proj_b[0:H]
    bk = in_proj_b[H:2 * H]
    bv = in_proj_b[2 * H:3 * H]
    owT = np.ascontiguousarray(out_w.T)
    w1T = np.ascontiguousarray(w1.T)                           # [2H, MD]
    w2T = np.ascontiguousarray(w2.T)                           # [MD, MD2]

    def cmaj(v):   # [H] -> [128, H/128] partition-major (column dt holds v[dt*128+p])
        return np.ascontiguousarray(v.reshape(-1, P).T)

    def bcast(v):  # [F] -> [128, F]
        return np.ascontiguousarray(np.broadcast_to(v[None, :], (P, v.shape[0])))

    shared = {
        "wqT": wqT, "wkT": wkT, "wvT": wvT,
        "bq_c": cmaj(bq), "bk_c": cmaj(bk), "bv_b": bcast(bv),
        "owT": owT, "ob_c": cmaj(out_b),
        "w1T": w1T, "b1_cd": cmaj(b1), "g1_cd": cmaj(g1), "be1_cd": cmaj(beta1),
        "w2T": w2T, "b2_b": bcast(b2), "g2_b": bcast(g2), "be2_b": bcast(beta2),
        "w3_b": bcast(w3[0]), "b3_c": np.full((P, 1), b3[0], dtype=np.float32),
        "table": np.ascontiguousarray(table[:, None]),
    }

    in_maps = []
    for c in range(8):
        b = c // 2
        half = c % 2
        own = slice(half * SQ, (half + 1) * SQ)
        oth = slice((1 - half) * SQ, (2 - half) * SQ)
        hT_b = hidden[b].T                                     # [H, S] view
        # arrange so own half occupies columns [0, SQ)
        hT_arr = np.ascontiguousarray(
            np.concatenate([hT_b[:, own], hT_b[:, oth]], axis=1))
        pT_arr = np.ascontiguousarray(
            np.concatenate([posT[:, own], posT[:, oth]], axis=1))
        kb = np.where(mask[b], 0.0, -1e9).astype(np.float32)
        kb_arr = np.concatenate([kb[own], kb[oth]])            # match column remap
        m = {
            "hT": hT_arr, "pT": pT_arr,
            "kbias": np.ascontiguousarray(kb_arr.reshape(-1, P).T),
            "maskf": np.ascontiguousarray(
                mask[b, own].astype(np.float32).reshape(-1, P).T),
            "tok": np.ascontiguousarray(token_ids[b, own][:, None]),
        }
        m.update(shared)
        in_maps.append(m)
    return in_maps


def _assemble(res):
    full = np.zeros((4, S), dtype=np.float32)
    for c in range(8):
        b = c // 2
        half = c % 2
        full[b, half * SQ:(half + 1) * SQ] = res.results[c]["out"]
    return full


def kernel(**inputs) -> np.ndarray:
    from concourse.bass_utils import run_bass_kernel_spmd
    in_maps = _prep_in_maps(inputs)
    nc = _get_program()
    res = run_bass_kernel_spmd(nc, in_maps, list(range(8)))
    return _assemble(res)


def run_traced(inputs, **kwargs):
    from concourse.bass_utils import run_bass_kernel_spmd
    in_maps = _prep_in_maps(inputs)
    nc = _get_program()
    return run_bass_kernel_spmd(nc, in_maps, list(range(8)), trace=True, **kwargs)



# revision 5
# speedup vs baseline: 1.2719x; 1.2719x over previous
"""Trainium2 Bass kernel for EnhancedMetaWeightNetwork (v2).

Full (unsharded) inputs in, full output out. 8 NeuronCores, core c handles
batch b = c // 2 and query-row half c % 2 (1024 query rows). K/V computed
per-core for the full sequence of the core's batch (duplicated across the
2 cores sharing a batch; no collectives).

v2 strategy (vs v1: fp32r + DRAM spills):
  - host precomputes x = hidden + pos, pre-transposed/sharded, in fp8
    (for QKV projections) and bf16 (for the meta-MLP x-part).
  - all activations SBUF-resident, no DRAM scratch round-trips.
  - fp8e4 DoubleRow matmuls (256-deep contraction, 2 cyc-per-4-rows) for
    Q/K/V projections, attention ctx, softmax denominator, out-proj.
    Weights pre-scaled into the fp8 dynamic range (wq,wk x64; wv x16;
    ow x64), descaling folded into free ACT scale immediates. Attention
    output is diluted ~40:1 in the meta-MLP input, so fp8 noise there is
    harmless to the 2e-2 gate.
  - scores matmul in bf16 ([keys,queries]-major), exp on ACT with the
    fp8 output written directly; per-head wide [128,1024] 2-bank PSUM
    score tiles amortize ACT access latency. Optional: route some key
    tiles' exp to DVE/Pool via the Schraudolph bit trick producing fp8
    bits directly (KB_DVE_KT / KB_POOL_KT env knobs).
  - softmax denominator via fp8-DoubleRow ones-matmul (free-rides on the
    packed ex tiles).
  - meta-MLP in bf16; h1's x-part runs before attention; LN2/final tail
    per-token-tile to keep it short.
"""

import os
import numpy as np

H = 1024
NH = 8
HD = 128           # head dim
S = 2048           # keys / full sequence
SQ = 1024          # own query rows per core
MD = 256           # meta dim
MD2 = 128
VOCAB = 32000
MIN_W, MAX_W = 0.1, 5.0
LN_EPS = 1e-5
P = 128

# fp8 scale plan
SW = 64.0          # wq, wk weight scale
SV = 16.0          # wv weight scale
SO = 64.0          # ow weight scale
CTX_S = 32.0       # ctx fp8 storage scale
EXP_SHIFT = 3.5    # exp(s - EXP_SHIFT): keeps ex in e4m3 range (max 240)
INV_SQRT_HD = 1.0 / np.sqrt(np.float32(HD))
SC_A = INV_SQRT_HD / (SW * SW)          # psum -> true scores
A8 = 8.0 / np.log(2.0)                   # fp8e4 bits per unit ln
DVE_A = A8 * SC_A

_CACHE = {}


def _build(dve_kt=0, pool_kt=0):
    """dve_kt / pool_kt: number of the 16 key-tiles per (head) whose exp
    is computed on DVE / Pool via the Schraudolph fp8 bit trick instead
    of ACT."""
    import concourse.bass as bass
    import concourse.mybir as mybir
    import concourse.tile as tile
    from concourse import bacc

    f32 = mybir.dt.float32
    bf16 = mybir.dt.bfloat16
    f8 = mybir.dt.float8e4
    u8 = mybir.dt.uint8
    i32 = mybir.dt.int32
    OP = mybir.AluOpType
    ACT = mybir.ActivationFunctionType
    DR = mybir.MatmulPerfMode.DoubleRow

    act_kt = 16 - dve_kt - pool_kt
    assert act_kt >= 0

    nc = bacc.Bacc("TRN2", target_bir_lowering=False, debug=False,
                   enable_asserts=False, num_devices=8)

    # ---------------- DRAM parameters (all host-prearranged) ----------------
    dp = nc.declare_dram_parameter
    x8d = dp("x8d", [P, 8 * S], f8, isOutput=False)       # x^T fp8 [p, c8, t] own-first
    xbd = dp("xbd", [P, 8 * SQ], bf16, isOutput=False)    # x^T bf16 own half
    wq8 = dp("wq8", [P, 8 * H], f8, isOutput=False)       # (in_proj_w[0:H]*SW).T packed
    wk8 = dp("wk8", [P, 8 * H], f8, isOutput=False)
    wv8 = dp("wv8", [P, 8 * H], f8, isOutput=False)
    ow8 = dp("ow8", [P, 8 * H], f8, isOutput=False)       # (out_w*SO).T packed
    w1bd = dp("w1bd", [P, 16 * MD], bf16, isOutput=False)  # w1.T packed bf16
    w2bd = dp("w2bd", [P, 2 * MD2], bf16, isOutput=False)  # w2.T packed bf16
    bq_c = dp("bq_c", [P, 8], f32, isOutput=False)        # SW * bias, partition-major
    bk_c = dp("bk_c", [P, 8], f32, isOutput=False)
    bv_b = dp("bv_b", [P, H], f32, isOutput=False)        # SV * bv broadcast rows
    ob_c = dp("ob_c", [P, 8], f32, isOutput=False)        # out bias (natural)
    b1_c = dp("b1_c", [P, 2], f32, isOutput=False)
    g1_c = dp("g1_c", [P, 2], f32, isOutput=False)
    be1_c = dp("be1_c", [P, 2], f32, isOutput=False)
    b2_b = dp("b2_b", [P, MD2], f32, isOutput=False)
    g2_b = dp("g2_b", [P, MD2], f32, isOutput=False)
    be2_b = dp("be2_b", [P, MD2], f32, isOutput=False)
    w3_b = dp("w3_b", [P, MD2], f32, isOutput=False)
    b3_c = dp("b3_c", [P, 1], f32, isOutput=False)
    kbias = dp("kbias", [P, 16], f32, isOutput=False)     # -EXP_SHIFT (+mask)
    dveB = dp("dveB", [P, 16], f32, isOutput=False)       # 56 - EXP_SHIFT*A8 (+mask)
    maskf = dp("maskf", [P, 8], f32, isOutput=False)
    tok = dp("tok", [SQ, 1], i32, isOutput=False)
    table = dp("table", [VOCAB, 1], f32, isOutput=False)
    out = dp("out", [SQ], f32, isOutput=True)

    with tile.TileContext(nc) as tc:
        with tc.tile_pool(name="const", bufs=1) as cst, \
             tc.tile_pool(name="main", bufs=1) as mn:

            # -------- long-lived SBUF --------
            x8 = mn.tile([P, 8, S], f8, tag="x8")
            for c8 in range(8):
                nc.sync.dma_start(x8[:, c8, :], x8d[:, c8 * S:(c8 + 1) * S])
            xb = mn.tile([P, 8, SQ], bf16, tag="xb")
            nc.sync.dma_start(xb[:], xbd[:].rearrange("p (c n) -> p c n", c=8))
            qT = mn.tile([P, 8, SQ], bf16, tag="qT")
            kT = mn.tile([P, 8, S], bf16, tag="kT")
            v8 = mn.tile([P, 16, H], f8, tag="v8")
            ctx8 = mn.tile([P, 8, SQ], f8, tag="ctx8")
            w1b = mn.tile([P, 16, MD], bf16, tag="w1b")
            nc.sync.dma_start(w1b[:], w1bd[:].rearrange("p (c n) -> p c n", c=16))
            h1x = mn.tile([P, 2, SQ], f32, tag="h1x")
            ow = mn.tile([P, 8, H], f8, tag="ow")

            # -------- constants / small params --------
            ones8 = cst.tile([P, 2, P], f8, tag="ones8")
            nc.any.memset(ones8[:], 1.0)
            onesb = cst.tile([P, P], bf16, tag="onesb")
            nc.any.memset(onesb[:], 1.0)
            eps_sb = cst.tile([P, 1], f32, tag="eps")
            nc.any.memset(eps_sb[:], LN_EPS)

            def cload(shape, tag, src, dt=f32):
                t = cst.tile(shape, dt, tag=tag)
                nc.sync.dma_start(t[:], src[:])
                return t

            kbias_sb = cload([P, 16], "kbias", kbias)
            dveB_sb = cload([P, 16], "dveB", dveB)
            maskf_sb = cload([P, 8], "maskf", maskf)
            bq_sb = cload([P, 8], "bq", bq_c)
            bk_sb = cload([P, 8], "bk", bk_c)
            bv_sb = cload([P, H], "bv", bv_b)
            ob_sb = cload([P, 8], "ob", ob_c)
            b1_sb = cload([P, 2], "b1", b1_c)
            g1_sb = cload([P, 2], "g1", g1_c)
            be1_sb = cload([P, 2], "be1", be1_c)
            b2_sb = cload([P, MD2], "b2", b2_b)
            g2_sb = cload([P, MD2], "g2", g2_b)
            be2_sb = cload([P, MD2], "be2", be2_b)
            w3_sb = cload([P, MD2], "w3", w3_b)
            b3_sb = cload([P, 1], "b3", b3_c)
            w2b = cload([P, 2 * MD2], "w2b", w2bd, dt=bf16)
            w2v = w2b[:].rearrange("p (c n) -> p c n", c=2)

            imp_all = cst.tile([P, 8], f32, tag="imp_all")
            for tt in range(8):
                itt = cst.tile([P, 1], i32, tag=f"it{tt}")
                nc.sync.dma_start(itt[:], tok[tt * P:(tt + 1) * P, :])
                nc.gpsimd.indirect_dma_start(
                    out=imp_all[:, tt:tt + 1], out_offset=None, in_=table[:],
                    in_offset=bass.IndirectOffsetOnAxis(ap=itt[:, :1], axis=0))

            # ================= QKV projections (fp8 DoubleRow) =================
            with tc.tile_pool(name="wqkv", bufs=1) as wp, \
                 tc.tile_pool(name="stg1", bufs=4) as stg, \
                 tc.tile_pool(name="ps1", bufs=6, space="PSUM") as ps1:
                wq = wp.tile([P, 8, H], f8, tag="wq")
                nc.sync.dma_start(wq[:], wq8[:].rearrange("p (c n) -> p c n", c=8))
                wk = wp.tile([P, 8, H], f8, tag="wk")
                nc.sync.dma_start(wk[:], wk8[:].rearrange("p (c n) -> p c n", c=8))
                wv = wp.tile([P, 8, H], f8, tag="wv")
                nc.sync.dma_start(wv[:], wv8[:].rearrange("p (c n) -> p c n", c=8))

                for dt in range(8):           # Q (own tokens only)
                    for qb in range(2):
                        psq = ps1.tile([P, 512], f32, tag="mm512", name="psq")
                        for c4 in range(4):
                            nc.tensor.matmul(
                                psq[:],
                                lhsT=wq[:, 2 * c4:2 * c4 + 2, dt * P:(dt + 1) * P],
                                rhs=x8[:, 2 * c4:2 * c4 + 2, qb * 512:(qb + 1) * 512],
                                perf_mode=DR, start=(c4 == 0), stop=(c4 == 3))
                        nc.scalar.activation(qT[:, dt, qb * 512:(qb + 1) * 512],
                                             psq[:], ACT.Identity,
                                             bias=bq_sb[:, dt:dt + 1], scale=1.0)
                for dt in range(8):           # K (full sequence)
                    for sb in range(4):
                        psk = ps1.tile([P, 512], f32, tag="mm512", name="psk")
                        for c4 in range(4):
                            nc.tensor.matmul(
                                psk[:],
                                lhsT=wk[:, 2 * c4:2 * c4 + 2, dt * P:(dt + 1) * P],
                                rhs=x8[:, 2 * c4:2 * c4 + 2, sb * 512:(sb + 1) * 512],
                                perf_mode=DR, start=(c4 == 0), stop=(c4 == 3))
                        nc.scalar.activation(kT[:, dt, sb * 512:(sb + 1) * 512],
                                             psk[:], ACT.Identity,
                                             bias=bk_sb[:, dt:dt + 1], scale=1.0)
                for tt in range(16):          # V (full sequence, token-major out)
                    for vb in range(2):
                        psv = ps1.tile([P, 512], f32, tag="mm512", name="psv")
                        for c4 in range(4):
                            nc.tensor.matmul(
                                psv[:],
                                lhsT=x8[:, 2 * c4:2 * c4 + 2, tt * P:(tt + 1) * P],
                                rhs=wv[:, 2 * c4:2 * c4 + 2, vb * 512:(vb + 1) * 512],
                                perf_mode=DR, start=(c4 == 0), stop=(c4 == 3))
                        nc.vector.tensor_tensor(
                            out=v8[:, tt, vb * 512:(vb + 1) * 512], in0=psv[:],
                            in1=bv_sb[:, vb * 512:(vb + 1) * 512], op=OP.add)

                # h1 x-part early (bf16), staged to f32 SBUF with b1 folded in
                for ft in range(2):
                    for qb in range(2):
                        psx = ps1.tile([P, 512], f32, tag="mm512", name="psx")
                        for c8 in range(8):
                            nc.tensor.matmul(
                                psx[:],
                                lhsT=w1b[:, c8, ft * P:(ft + 1) * P],
                                rhs=xb[:, c8, qb * 512:(qb + 1) * 512],
                                start=(c8 == 0), stop=(c8 == 7))
                        nc.scalar.activation(h1x[:, ft, qb * 512:(qb + 1) * 512],
                                             psx[:], ACT.Identity,
                                             bias=b1_sb[:, ft:ft + 1], scale=1.0)

            # ================= attention =================
            with tc.tile_pool(name="exp", bufs=2) as exp_pool, \
                 tc.tile_pool(name="rcp", bufs=2) as rcp, \
                 tc.tile_pool(name="ps_sc", bufs=2, space="PSUM") as ps_sc, \
                 tc.tile_pool(name="ps_cd", bufs=4, space="PSUM") as ps_cd:
                nc.sync.dma_start(ow[:], ow8[:].rearrange("p (c n) -> p c n", c=8))

                for h in range(NH):
                    cps0 = ps_cd.tile([P, 512], f32, tag="cd", name="cps0")
                    cps1 = ps_cd.tile([P, 512], f32, tag="cd", name="cps1")
                    dns0 = ps_cd.tile([P, 512], f32, tag="cd", name="dns0")
                    dns1 = ps_cd.tile([P, 512], f32, tag="cd", name="dns1")
                    exs = []
                    for kt in range(16):
                        sc = ps_sc.tile([P, SQ], f32, tag="sc")  # 2 banks
                        for qb in range(2):
                            nc.tensor.matmul(
                                sc[:, qb * 512:(qb + 1) * 512],
                                lhsT=kT[:, h, kt * P:(kt + 1) * P],
                                rhs=qT[:, h, qb * 512:(qb + 1) * 512],
                                start=True, stop=True)
                        if kt % 2 == 0:
                            ex = exp_pool.tile([P, 2, SQ], f8, tag="ex")
                            exs.append(ex)
                        exd = ex[:, kt % 2, :]
                        if kt < act_kt:
                            nc.scalar.activation(exd, sc[:], ACT.Exp,
                                                 bias=kbias_sb[:, kt:kt + 1],
                                                 scale=SC_A)
                        else:
                            eng = nc.vector if kt < act_kt + dve_kt else nc.gpsimd
                            eng.scalar_tensor_tensor(
                                out=exd.bitcast(u8), in0=sc[:], scalar=DVE_A,
                                in1=dveB_sb[:, kt:kt + 1].to_broadcast([P, SQ]),
                                op0=OP.mult, op1=OP.add)
                        if kt % 2 == 1:
                            pair = kt // 2
                            st, sp = (pair == 0), (pair == 7)
                            for qb, (cp, dn) in enumerate(((cps0, dns0),
                                                           (cps1, dns1))):
                                nc.tensor.matmul(
                                    cp[:], lhsT=v8[:, kt - 1:kt + 1,
                                                   h * P:(h + 1) * P],
                                    rhs=ex[:, :, qb * 512:(qb + 1) * 512],
                                    perf_mode=DR, start=st, stop=sp)
                                nc.tensor.matmul(
                                    dn[:], lhsT=ones8[:],
                                    rhs=ex[:, :, qb * 512:(qb + 1) * 512],
                                    perf_mode=DR, start=st, stop=sp)
                    for qb, (cp, dn) in enumerate(((cps0, dns0), (cps1, dns1))):
                        rcb = rcp.tile([P, 512], f32, tag="rcb")
                        with nc.allow_low_precision(reason="softmax recip"):
                            nc.vector.reciprocal(rcb[:], dn[:])
                        nc.vector.scalar_tensor_tensor(
                            out=ctx8[:, h, qb * 512:(qb + 1) * 512],
                            in0=cp[:], scalar=CTX_S / SV, in1=rcb[:],
                            op0=OP.mult, op1=OP.mult)

            # ================= out-proj + meta MLP =================
            with tc.tile_pool(name="attp", bufs=1) as attp, \
                 tc.tile_pool(name="h1p_", bufs=1) as h1pp, \
                 tc.tile_pool(name="sml", bufs=1) as sml, \
                 tc.tile_pool(name="ps2", bufs=6, space="PSUM") as ps2:
                att_b = attp.tile([P, 8, SQ], bf16, tag="att_b")
                h1p = h1pp.tile([P, 2, SQ], bf16, tag="h1p")
                h1sq = h1pp.tile([P, 2, SQ], bf16, tag="h1sq")
                h1n = h1pp.tile([P, 2, SQ], bf16, tag="h1n")
                stat = h1pp.tile([P, 2, SQ], f32, tag="stat")
                nmean, work = stat[:, 0, :], stat[:, 1, :]
                res_sb = h1pp.tile([P, 8], f32, tag="res")
                hb2_all = h1pp.tile([P, 8, MD2], f32, tag="hb2_all")

                for qb in range(2):
                    qsl = slice(qb * 512, (qb + 1) * 512)
                    # out-projection (fp8 DR over ctx features)
                    for dt in range(8):
                        pso = ps2.tile([P, 512], f32, tag="mm512", name="pso")
                        for c4 in range(4):
                            nc.tensor.matmul(
                                pso[:],
                                lhsT=ow[:, 2 * c4:2 * c4 + 2, dt * P:(dt + 1) * P],
                                rhs=ctx8[:, 2 * c4:2 * c4 + 2, qsl],
                                perf_mode=DR, start=(c4 == 0), stop=(c4 == 3))
                        nc.scalar.activation(att_b[:, dt, qsl], pso[:],
                                             ACT.Identity,
                                             bias=ob_sb[:, dt:dt + 1],
                                             scale=1.0 / (CTX_S * SO))
                    # h1 attention part (bf16) + combine with early x part
                    for ft in range(2):
                        psa = ps2.tile([P, 512], f32, tag="mm512", name="psa")
                        for c8 in range(8):
                            nc.tensor.matmul(
                                psa[:],
                                lhsT=w1b[:, 8 + c8, ft * P:(ft + 1) * P],
                                rhs=att_b[:, c8, qsl],
                                start=(c8 == 0), stop=(c8 == 7))
                        nc.vector.tensor_tensor(out=h1p[:, ft, qsl], in0=psa[:],
                                                in1=h1x[:, ft, qsl], op=OP.add)
                        nc.vector.tensor_tensor(out=h1sq[:, ft, qsl],
                                                in0=h1p[:, ft, qsl],
                                                in1=h1p[:, ft, qsl], op=OP.mult)
                    # LN1 stats via ones-matmul column sums
                    psA = ps2.tile([P, 512], f32, tag="mm512", name="psA")
                    psB = ps2.tile([P, 512], f32, tag="mm512", name="psB")
                    for ft in range(2):
                        nc.tensor.matmul(psA[:], lhsT=onesb[:],
                                         rhs=h1p[:, ft, qsl],
                                         start=(ft == 0), stop=(ft == 1))
                    for ft in range(2):
                        nc.tensor.matmul(psB[:], lhsT=onesb[:],
                                         rhs=h1sq[:, ft, qsl],
                                         start=(ft == 0), stop=(ft == 1))
                    nc.vector.tensor_scalar_mul(nmean[:, qsl], psA[:], -1.0 / MD)
                    nc.vector.tensor_scalar_mul(work[:, qsl], psB[:], 1.0 / MD)
                    m2 = sml.tile([P, 512], f32, tag="m2")
                    nc.vector.tensor_tensor(out=m2[:], in0=nmean[:, qsl],
                                            in1=nmean[:, qsl], op=OP.mult)
                    nc.vector.tensor_tensor(out=work[:, qsl], in0=work[:, qsl],
                                            in1=m2[:], op=OP.subtract)
                    # rstd = 1/sqrt(var + eps)
                    std = sml.tile([P, 512], f32, tag="std")
                    nc.scalar.activation(std[:], work[:, qsl], ACT.Sqrt,
                                         bias=eps_sb[:, 0:1], scale=1.0)
                    with nc.allow_low_precision(reason="ln1 recip"):
                        nc.vector.reciprocal(work[:, qsl], std[:])
                    for ft in range(2):
                        nc.vector.tensor_tensor(out=h1n[:, ft, qsl],
                                                in0=h1p[:, ft, qsl],
                                                in1=nmean[:, qsl], op=OP.add)
                        nc.vector.tensor_tensor(out=h1n[:, ft, qsl],
                                                in0=h1n[:, ft, qsl],
                                                in1=work[:, qsl], op=OP.mult)
                        nc.scalar.activation(h1n[:, ft, qsl], h1n[:, ft, qsl],
                                             ACT.Relu, bias=be1_sb[:, ft:ft + 1],
                                             scale=g1_sb[:, ft:ft + 1])

                # h2 + LN2 + final, per token tile
                F2 = float(MD2)
                for tt in range(8):
                    ph2 = ps2.tile([P, 512], f32, tag="mm512", name="ph2")
                    for ft in range(2):
                        nc.tensor.matmul(ph2[:, :MD2],
                                         lhsT=h1n[:, ft, tt * P:(tt + 1) * P],
                                         rhs=w2v[:, ft, :],
                                         start=(ft == 0), stop=(ft == 1))
                    nc.vector.tensor_tensor(out=hb2_all[:, tt, :],
                                            in0=ph2[:, :MD2], in1=b2_sb[:],
                                            op=OP.add)
                sums2 = sml.tile([P, 8], f32, tag="sums2")
                nc.vector.reduce_sum(sums2[:], hb2_all[:],
                                     axis=mybir.AxisListType.X)
                msq = sml.tile([P, 8, MD2], f32, tag="msq")
                nc.vector.tensor_tensor(out=msq[:], in0=hb2_all[:],
                                        in1=hb2_all[:], op=OP.mult)
                ssq2 = sml.tile([P, 8], f32, tag="ssq2")
                nc.vector.reduce_sum(ssq2[:], msq[:], axis=mybir.AxisListType.X)
                nm2 = sml.tile([P, 8], f32, tag="nm2")
                nc.vector.tensor_scalar_mul(nm2[:], sums2[:], -1.0 / F2)
                ex22 = sml.tile([P, 8], f32, tag="ex22")
                nc.vector.tensor_scalar_mul(ex22[:], ssq2[:], 1.0 / F2)
                mm2 = sml.tile([P, 8], f32, tag="mm2")
                nc.vector.tensor_tensor(out=mm2[:], in0=nm2[:], in1=nm2[:],
                                        op=OP.mult)
                var2 = sml.tile([P, 8], f32, tag="var2")
                nc.vector.tensor_tensor(out=var2[:], in0=ex22[:], in1=mm2[:],
                                        op=OP.subtract)
                std2 = sml.tile([P, 8], f32, tag="std2")
                nc.scalar.activation(std2[:], var2[:], ACT.Sqrt,
                                     bias=eps_sb[:, 0:1], scale=1.0)
                rstd2 = sml.tile([P, 8], f32, tag="rstd2")
                with nc.allow_low_precision(reason="ln2 recip"):
                    nc.vector.reciprocal(rstd2[:], std2[:])
                t1a = sml.tile([P, 8, MD2], f32, tag="t1a")
                nc.vector.tensor_tensor(
                    out=t1a[:], in0=hb2_all[:],
                    in1=nm2[:, :, None].to_broadcast([P, 8, MD2]), op=OP.add)
                nc.vector.tensor_tensor(
                    out=t1a[:], in0=t1a[:],
                    in1=rstd2[:, :, None].to_broadcast([P, 8, MD2]), op=OP.mult)
                nc.vector.tensor_tensor(
                    out=t1a[:], in0=t1a[:],
                    in1=g2_sb[:, None, :].to_broadcast([P, 8, MD2]), op=OP.mult)
                nc.vector.tensor_tensor(
                    out=t1a[:], in0=t1a[:],
                    in1=be2_sb[:, None, :].to_broadcast([P, 8, MD2]), op=OP.add)
                nc.vector.tensor_scalar_max(t1a[:], t1a[:], 0.0)
                nc.vector.tensor_tensor(
                    out=t1a[:], in0=t1a[:],
                    in1=w3_sb[:, None, :].to_broadcast([P, 8, MD2]), op=OP.mult)
                base8 = sml.tile([P, 8], f32, tag="base8")
                nc.vector.reduce_sum(base8[:], t1a[:], axis=mybir.AxisListType.X)
                nc.vector.tensor_tensor(
                    out=base8[:], in0=base8[:],
                    in1=b3_sb[:, 0:1].to_broadcast([P, 8]), op=OP.add)
                imp1a = sml.tile([P, 8], f32, tag="imp1a")
                nc.vector.tensor_scalar_add(imp1a[:], imp_all[:], 1.0)
                nc.vector.tensor_tensor(out=base8[:], in0=base8[:],
                                        in1=imp1a[:], op=OP.mult)
                nc.vector.tensor_scalar(base8[:], base8[:], MAX_W, MIN_W,
                                        op0=OP.min, op1=OP.max)
                nc.vector.tensor_tensor(out=res_sb[:], in0=base8[:],
                                        in1=maskf_sb[:], op=OP.mult)
                nc.sync.dma_start(out[:].rearrange("(t p) -> p t", p=P),
                                  res_sb[:])

    nc.compile()
    return nc


def _get_program():
    dve_kt = int(os.environ.get("KB_DVE_KT", "0"))
    pool_kt = int(os.environ.get("KB_POOL_KT", "0"))
    key = ("nc", dve_kt, pool_kt)
    if key not in _CACHE:
        _CACHE[key] = _build(dve_kt, pool_kt)
    return _CACHE[key]


def _pack8(mat):
    """[R, C] with R = 8k*128 -> [128, 8k*C] chunk-major fp8-ready layout."""
    import ml_dtypes
    r, c = mat.shape
    nchunk = r // P
    return np.ascontiguousarray(
        mat.reshape(nchunk, P, c).transpose(1, 0, 2).reshape(P, nchunk * c))


def _prep_in_maps(inputs):
    import ml_dtypes
    f8 = ml_dtypes.float8_e4m3
    bf16 = ml_dtypes.bfloat16

    hidden = np.asarray(inputs["hidden_states"], dtype=np.float32)
    token_ids = np.asarray(inputs["token_ids"], dtype=np.int32)
    mask = np.asarray(inputs["attention_mask"]).astype(bool)
    pos = np.asarray(inputs["pos_embed"], dtype=np.float32)
    in_proj_w = np.asarray(inputs["in_proj_w"], dtype=np.float32)
    in_proj_b = np.asarray(inputs["in_proj_b"], dtype=np.float32)
    out_w = np.asarray(inputs["out_w"], dtype=np.float32)
    out_b = np.asarray(inputs["out_b"], dtype=np.float32)
    w1 = np.asarray(inputs["w1"], dtype=np.float32)
    b1 = np.asarray(inputs["b1"], dtype=np.float32)
    g1 = np.asarray(inputs["g1"], dtype=np.float32)
    beta1 = np.asarray(inputs["beta1"], dtype=np.float32)
    w2 = np.asarray(inputs["w2"], dtype=np.float32)
    b2 = np.asarray(inputs["b2"], dtype=np.float32)
    g2 = np.asarray(inputs["g2"], dtype=np.float32)
    beta2 = np.asarray(inputs["beta2"], dtype=np.float32)
    w3 = np.asarray(inputs["w3"], dtype=np.float32)
    b3 = np.asarray(inputs["b3"], dtype=np.float32)
    table = np.asarray(inputs["importance_table"], dtype=np.float32)

    B, S_, H_ = hidden.shape
    assert (B, S_, H_) == (4, S, H), (B, S_, H_)

    x = hidden + pos[:, :S, :]                             # [B, S, H]

    def cmaj(v):   # [8*128] -> [128, 8] partition-major
        return np.ascontiguousarray(v.reshape(-1, P).T)

    def bcast(v):  # [F] -> [128, F]
        return np.ascontiguousarray(np.broadcast_to(v[None, :], (P, v.shape[0])))

    wq8 = _pack8((in_proj_w[0:H] * SW).T).astype(f8)
    wk8 = _pack8((in_proj_w[H:2 * H] * SW).T).astype(f8)
    wv8 = _pack8((in_proj_w[2 * H:3 * H] * SV).T).astype(f8)
    ow8 = _pack8((out_w * SO).T).astype(f8)
    w1b = _pack8(w1.T).astype(bf16)                        # [2H, MD] packed
    w2b = _pack8(w2.T).astype(bf16)                        # [MD, MD2] packed

    shared = {
        "wq8": wq8, "wk8": wk8, "wv8": wv8, "ow8": ow8,
        "w1bd": w1b, "w2bd": w2b,
        "bq_c": cmaj(in_proj_b[0:H] * SW).astype(np.float32),
        "bk_c": cmaj(in_proj_b[H:2 * H] * SW).astype(np.float32),
        "bv_b": bcast(in_proj_b[2 * H:3 * H] * SV).astype(np.float32),
        "ob_c": cmaj(out_b).astype(np.float32),
        "b1_c": cmaj(b1), "g1_c": cmaj(g1), "be1_c": cmaj(beta1),
        "b2_b": bcast(b2), "g2_b": bcast(g2), "be2_b": bcast(beta2),
        "w3_b": bcast(w3[0]), "b3_c": np.full((P, 1), b3[0], dtype=np.float32),
        "table": np.ascontiguousarray(table[:, None]),
    }

    in_maps = []
    for c in range(8):
        b = c // 2
        half = c % 2
        own = slice(half * SQ, (half + 1) * SQ)
        oth = slice((1 - half) * SQ, (2 - half) * SQ)
        xT = x[b].T                                        # [H, S]
        xT_arr = np.concatenate([xT[:, own], xT[:, oth]], axis=1)
        mb = np.where(mask[b], 0.0, -1e9).astype(np.float32)
        mb_arr = np.concatenate([mb[own], mb[oth]])        # key-order remap
        kb = (mb_arr - EXP_SHIFT).reshape(16, P).T         # [128, 16]
        dB = (mb_arr * A8 + (56.0 - EXP_SHIFT * A8)).reshape(16, P).T
        m = {
            "x8d": _pack8(xT_arr).astype(f8),
            "xbd": _pack8(np.ascontiguousarray(xT_arr[:, :SQ])).astype(bf16),
            "kbias": np.ascontiguousarray(kb),
            "dveB": np.ascontiguousarray(dB),
            "maskf": np.ascontiguousarray(
                mask[b, own].astype(np.float32).reshape(-1, P).T),
            "tok": np.ascontiguousarray(token_ids[b, own][:, None]),
        }
        m.update(shared)
        in_maps.append(m)
    return in_maps


def _assemble(res):
    full = np.zeros((4, S), dtype=np.float32)
    for c in range(8):
        b = c // 2
        half = c % 2
        full[b, half * SQ:(half + 1) * SQ] = res.results[c]["out"]
    return full


def kernel(**inputs) -> np.ndarray:
    from concourse.bass_utils import run_bass_kernel_spmd
    in_maps = _prep_in_maps(inputs)
    nc = _get_program()
    res = run_bass_kernel_spmd(nc, in_maps, list(range(8)))
    return _assemble(res)


def run_traced(inputs, **kwargs):
    from concourse.bass_utils import run_bass_kernel_spmd
    in_maps = _prep_in_maps(inputs)
    nc = _get_program()
    return run_bass_kernel_spmd(nc, in_maps, list(range(8)), trace=True, **kwargs)


# revision 10
# speedup vs baseline: 1.4369x; 1.1297x over previous
"""Trainium2 Bass kernel for EnhancedMetaWeightNetwork (v2.1).

Full (unsharded) inputs in, full output out. 8 NeuronCores, core c handles
batch b = c // 2 and query-row half c % 2 (1024 query rows). K/V computed
per-core for the full sequence of the core's batch (duplicated across the
2 cores sharing a batch; no collectives).

Strategy:
  - host precomputes x = hidden + pos, pre-transposed/sharded, in fp8
    (for QKV projections) and bf16 (for the meta-MLP x-part).
  - all activations SBUF-resident, no DRAM scratch round-trips.
  - fp8e4 DoubleRow matmuls (256-deep contraction) for Q/K/V projections,
    attention ctx, softmax denominator, out-proj. Weights pre-scaled into
    the fp8 dynamic range (wq,wk x64; wv x16; ow x64), descaling folded
    into ACT scale immediates. Attention output is diluted ~40:1 in the
    meta-MLP input, so fp8 noise there is harmless to the 2e-2 gate.
  - per-matmul overhead (~120ns ldweights/dispatch) dominates short
    matmuls, so everything streams 1024-wide into 2-bank PSUM tiles.
  - scores matmul in bf16 ([keys, queries]-major; one 1024-wide matmul
    per key tile); exp on ACT writing fp8 directly; optional: route some
    key tiles' exp to DVE/Pool via the Schraudolph bit trick producing
    fp8 bits (KB_DVE_KT / KB_POOL_KT env knobs).
  - softmax 1/denominator via magic-constant bit trick on DVE (one int
    subtract; ~5% err on a per-query scale, diluted to <0.2% final) —
    the native DVE RECIPROCAL is ~6ns/elem.
  - meta-MLP in bf16; h1's x-part runs before attention; LN1 rstd via
    exp(-0.5*ln(var+eps)) on ACT (Exp table already resident).
"""

import os
import numpy as np

H = 1024
NH = 8
HD = 128           # head dim
S = 2048           # keys / full sequence
SQ = 1024          # own query rows per core
MD = 256           # meta dim
MD2 = 128
VOCAB = 32000
MIN_W, MAX_W = 0.1, 5.0
LN_EPS = 1e-5
P = 128

# fp8 scale plan
SW = 64.0          # wq, wk weight scale
SV = 16.0          # wv weight scale
SO = 64.0          # ow weight scale
CTX_S = 32.0       # ctx fp8 storage scale
EXP_SHIFT = 3.5    # exp(s - EXP_SHIFT): keeps ex in e4m3 range (max 240)
INV_SQRT_HD = 1.0 / np.sqrt(np.float32(HD))
SC_A = INV_SQRT_HD / (SW * SW)          # psum -> true scores
A8 = 8.0 / np.log(2.0)                   # fp8e4 bits per unit ln
DVE_A = A8 * SC_A
RCP_MAGIC_F32 = np.int32(0x7EF311C0).view(np.float32) if hasattr(np.int32(0), 'view') else None

_CACHE = {}


def _build(dve_kt=0, pool_kt=0):
    """dve_kt / pool_kt: number of the 16 key-tiles per head whose exp is
    computed on DVE / Pool via the Schraudolph fp8 bit trick instead of
    ACT."""
    import concourse.bass as bass
    import concourse.mybir as mybir
    import concourse.tile as tile
    from concourse import bacc

    f32 = mybir.dt.float32
    bf16 = mybir.dt.bfloat16
    f8 = mybir.dt.float8e4
    u8 = mybir.dt.uint8
    i32 = mybir.dt.int32
    OP = mybir.AluOpType
    ACT = mybir.ActivationFunctionType
    DR = mybir.MatmulPerfMode.DoubleRow

    act_kt = 16 - dve_kt - pool_kt
    assert act_kt >= 0

    nc = bacc.Bacc("TRN2", target_bir_lowering=False, debug=False,
                   enable_asserts=False, num_devices=8)

    # ---------------- DRAM parameters (all host-prearranged) ----------------
    dp = nc.declare_dram_parameter
    x8d = dp("x8d", [P, 8 * S], f8, isOutput=False)       # x^T fp8 [p, c8, t] own-first
    xbd = dp("xbd", [P, 8 * SQ], bf16, isOutput=False)    # x^T bf16 own half
    wq8 = dp("wq8", [P, 8 * H], f8, isOutput=False)       # (in_proj_w[0:H]*SW).T packed
    wk8 = dp("wk8", [P, 8 * H], f8, isOutput=False)
    wv8 = dp("wv8", [P, 8 * H], f8, isOutput=False)
    ow8 = dp("ow8", [P, 8 * H], f8, isOutput=False)       # (out_w*SO).T packed
    w1bd = dp("w1bd", [P, 16 * MD], bf16, isOutput=False)  # w1.T packed bf16
    w2bd = dp("w2bd", [P, 2 * MD2], bf16, isOutput=False)  # w2.T packed bf16
    bq_c = dp("bq_c", [P, 8], f32, isOutput=False)        # SW * bias, partition-major
    bk_c = dp("bk_c", [P, 8], f32, isOutput=False)
    bv_b = dp("bv_b", [P, H], f32, isOutput=False)        # SV * bv broadcast rows
    ob_c = dp("ob_c", [P, 8], f32, isOutput=False)        # out bias (natural)
    b1_c = dp("b1_c", [P, 2], f32, isOutput=False)
    g1_c = dp("g1_c", [P, 2], f32, isOutput=False)
    be1_c = dp("be1_c", [P, 2], f32, isOutput=False)
    b2_b = dp("b2_b", [P, MD2], f32, isOutput=False)
    g2_b = dp("g2_b", [P, MD2], f32, isOutput=False)
    be2_b = dp("be2_b", [P, MD2], f32, isOutput=False)
    w3_b = dp("w3_b", [P, MD2], f32, isOutput=False)
    b3_c = dp("b3_c", [P, 1], f32, isOutput=False)
    kbias = dp("kbias", [P, 16], f32, isOutput=False)     # -EXP_SHIFT (+mask)
    dveB = dp("dveB", [P, 16], f32, isOutput=False)       # 56 - EXP_SHIFT*A8 (+mask)
    maskf = dp("maskf", [P, 8], f32, isOutput=False)
    tok = dp("tok", [SQ, 1], i32, isOutput=False)
    table = dp("table", [VOCAB, 1], f32, isOutput=False)
    out = dp("out", [SQ], f32, isOutput=True)

    with tile.TileContext(nc) as tc:
        with tc.tile_pool(name="const", bufs=1) as cst, \
             tc.tile_pool(name="main", bufs=1) as mn:

            # -------- long-lived SBUF --------
            x8 = mn.tile([P, 8, S], f8, tag="x8")
            for c8 in range(8):
                nc.sync.dma_start(x8[:, c8, :], x8d[:, c8 * S:(c8 + 1) * S])
            xb = mn.tile([P, 8, SQ], bf16, tag="xb")
            nc.sync.dma_start(xb[:], xbd[:].rearrange("p (c n) -> p c n", c=8))
            qT = mn.tile([P, 8, SQ], bf16, tag="qT")
            kT = mn.tile([P, 8, S], bf16, tag="kT")
            v8 = mn.tile([P, 16, H], f8, tag="v8")
            ctx8 = mn.tile([P, 8, SQ], f8, tag="ctx8")
            w1b = mn.tile([P, 16, MD], bf16, tag="w1b")
            nc.sync.dma_start(w1b[:], w1bd[:].rearrange("p (c n) -> p c n", c=16))
            h1x = mn.tile([P, 2, SQ], f32, tag="h1x")
            ow = mn.tile([P, 8, H], f8, tag="ow")

            # -------- constants / small params --------
            ones8 = cst.tile([P, 2, P], f8, tag="ones8")
            nc.any.memset(ones8[:], 1.0)
            onesb = cst.tile([P, P], bf16, tag="onesb")
            nc.any.memset(onesb[:], 1.0)
            eps_sb = cst.tile([P, 1], f32, tag="eps")
            nc.any.memset(eps_sb[:], LN_EPS)
            magicw = cst.tile([P, SQ], f32, tag="magicw")
            nc.any.memset(magicw[:], float(RCP_MAGIC_F32))

            def cload(shape, tag, src, dt=f32):
                t = cst.tile(shape, dt, tag=tag)
                nc.sync.dma_start(t[:], src[:])
                return t

            kbias_sb = cload([P, 16], "kbias", kbias)
            dveB_sb = cload([P, 16], "dveB", dveB)
            maskf_sb = cload([P, 8], "maskf", maskf)
            bq_sb = cload([P, 8], "bq", bq_c)
            bk_sb = cload([P, 8], "bk", bk_c)
            bv_sb = cload([P, H], "bv", bv_b)
            ob_sb = cload([P, 8], "ob", ob_c)
            b1_sb = cload([P, 2], "b1", b1_c)
            g1_sb = cload([P, 2], "g1", g1_c)
            be1_sb = cload([P, 2], "be1", be1_c)
            b2_sb = cload([P, MD2], "b2", b2_b)
            g2_sb = cload([P, MD2], "g2", g2_b)
            be2_sb = cload([P, MD2], "be2", be2_b)
            w3_sb = cload([P, MD2], "w3", w3_b)
            b3_sb = cload([P, 1], "b3", b3_c)
            w2b = cload([P, 2 * MD2], "w2b", w2bd, dt=bf16)
            w2v = w2b[:].rearrange("p (c n) -> p c n", c=2)

            imp_all = cst.tile([P, 8], f32, tag="imp_all")
            for tt in range(8):
                itt = cst.tile([P, 1], i32, tag=f"it{tt}")
                nc.sync.dma_start(itt[:], tok[tt * P:(tt + 1) * P, :])
                nc.gpsimd.indirect_dma_start(
                    out=imp_all[:, tt:tt + 1], out_offset=None, in_=table[:],
                    in_offset=bass.IndirectOffsetOnAxis(ap=itt[:, :1], axis=0))

            # ================= QKV projections (fp8 DoubleRow) =================
            with tc.tile_pool(name="wqkv", bufs=1) as wp, \
                 tc.tile_pool(name="ps1", bufs=4, space="PSUM") as ps1:
                wq = wp.tile([P, 8, H], f8, tag="wq")
                nc.sync.dma_start(wq[:], wq8[:].rearrange("p (c n) -> p c n", c=8))
                wk = wp.tile([P, 8, H], f8, tag="wk")
                nc.sync.dma_start(wk[:], wk8[:].rearrange("p (c n) -> p c n", c=8))
                wv = wp.tile([P, 8, H], f8, tag="wv")
                nc.sync.dma_start(wv[:], wv8[:].rearrange("p (c n) -> p c n", c=8))

                for dt in range(8):           # Q (own tokens only)
                    psq = ps1.tile([P, SQ], f32, tag="mmw", name="psq")
                    for hb in range(2):
                        for c4 in range(4):
                            nc.tensor.matmul(
                                psq[:, hb * 512:(hb + 1) * 512],
                                lhsT=wq[:, 2 * c4:2 * c4 + 2, dt * P:(dt + 1) * P],
                                rhs=x8[:, 2 * c4:2 * c4 + 2,
                                       hb * 512:(hb + 1) * 512],
                                perf_mode=DR, start=(c4 == 0), stop=(c4 == 3))
                    nc.scalar.activation(qT[:, dt, :], psq[:], ACT.Identity,
                                         bias=bq_sb[:, dt:dt + 1], scale=1.0)
                for dt in range(8):           # K (full sequence)
                    for sb in range(2):
                        psk = ps1.tile([P, SQ], f32, tag="mmw", name="psk")
                        for hb in range(2):
                            for c4 in range(4):
                                nc.tensor.matmul(
                                    psk[:, hb * 512:(hb + 1) * 512],
                                    lhsT=wk[:, 2 * c4:2 * c4 + 2,
                                            dt * P:(dt + 1) * P],
                                    rhs=x8[:, 2 * c4:2 * c4 + 2,
                                           sb * SQ + hb * 512:
                                           sb * SQ + (hb + 1) * 512],
                                    perf_mode=DR, start=(c4 == 0), stop=(c4 == 3))
                        nc.scalar.activation(kT[:, dt, sb * SQ:(sb + 1) * SQ],
                                             psk[:], ACT.Identity,
                                             bias=bk_sb[:, dt:dt + 1], scale=1.0)
                for tt in range(16):          # V (full sequence, token-major out)
                    psv = ps1.tile([P, SQ], f32, tag="mmw", name="psv")
                    for hb in range(2):
                        for c4 in range(4):
                            nc.tensor.matmul(
                                psv[:, hb * 512:(hb + 1) * 512],
                                lhsT=x8[:, 2 * c4:2 * c4 + 2, tt * P:(tt + 1) * P],
                                rhs=wv[:, 2 * c4:2 * c4 + 2,
                                       hb * 512:(hb + 1) * 512],
                                perf_mode=DR, start=(c4 == 0), stop=(c4 == 3))
                    nc.vector.tensor_tensor(out=v8[:, tt, :], in0=psv[:],
                                            in1=bv_sb[:], op=OP.add)

                # h1 x-part early (bf16), staged to f32 SBUF with b1 folded in
                for ft in range(2):
                    psx = ps1.tile([P, SQ], f32, tag="mmw", name="psx")
                    for hb in range(2):
                        for c8 in range(8):
                            nc.tensor.matmul(
                                psx[:, hb * 512:(hb + 1) * 512],
                                lhsT=w1b[:, c8, ft * P:(ft + 1) * P],
                                rhs=xb[:, c8, hb * 512:(hb + 1) * 512],
                                start=(c8 == 0), stop=(c8 == 7))
                    nc.scalar.activation(h1x[:, ft, :], psx[:], ACT.Identity,
                                         bias=b1_sb[:, ft:ft + 1], scale=1.0)

            # ================= attention =================
            with tc.tile_pool(name="exp", bufs=2) as exp_pool, \
                 tc.tile_pool(name="rcp", bufs=2) as rcp, \
                 tc.tile_pool(name="ps_sc", bufs=2, space="PSUM") as ps_sc, \
                 tc.tile_pool(name="ps_cd", bufs=2, space="PSUM") as ps_cd:
                nc.sync.dma_start(ow[:], ow8[:].rearrange("p (c n) -> p c n", c=8))

                for h in range(NH):
                    cps = ps_cd.tile([P, SQ], f32, tag="cd", name="cps")
                    dns = ps_cd.tile([P, SQ], f32, tag="cd", name="dns")
                    for kt in range(16):
                        sc = ps_sc.tile([P, SQ], f32, tag="sc")  # 2 banks
                        for hb in range(2):
                            nc.tensor.matmul(sc[:, hb * 512:(hb + 1) * 512],
                                             lhsT=kT[:, h, kt * P:(kt + 1) * P],
                                             rhs=qT[:, h, hb * 512:(hb + 1) * 512],
                                             start=True, stop=True)
                        if kt % 2 == 0:
                            ex = exp_pool.tile([P, 2, SQ], f8, tag="ex")
                        exd = ex[:, kt % 2, :]
                        if kt < act_kt:
                            nc.scalar.activation(exd, sc[:], ACT.Exp,
                                                 bias=kbias_sb[:, kt:kt + 1],
                                                 scale=SC_A)
                        else:
                            eng = nc.vector if kt < act_kt + dve_kt else nc.gpsimd
                            eng.scalar_tensor_tensor(
                                out=exd.bitcast(u8), in0=sc[:], scalar=DVE_A,
                                in1=dveB_sb[:, kt:kt + 1].to_broadcast([P, SQ]),
                                op0=OP.mult, op1=OP.add)
                        if kt % 2 == 1:
                            pair = kt // 2
                            st, sp = (pair == 0), (pair == 7)
                            for hb in range(2):
                                hsl = slice(hb * 512, (hb + 1) * 512)
                                nc.tensor.matmul(
                                    cps[:, hsl], lhsT=v8[:, kt - 1:kt + 1,
                                                         h * P:(h + 1) * P],
                                    rhs=ex[:, :, hsl], perf_mode=DR,
                                    start=st, stop=sp)
                                nc.tensor.matmul(
                                    dns[:, hsl], lhsT=ones8[:],
                                    rhs=ex[:, :, hsl], perf_mode=DR,
                                    start=st, stop=sp)
                    # 1/dns via magic-constant bit trick (one DVE int sub)
                    rcb = rcp.tile([P, SQ], f32, tag="rcb")
                    nc.vector.tensor_tensor(out=rcb[:].bitcast(i32),
                                            in0=magicw[:].bitcast(i32),
                                            in1=dns[:].bitcast(i32),
                                            op=OP.subtract)
                    nc.vector.scalar_tensor_tensor(
                        out=ctx8[:, h, :], in0=cps[:], scalar=CTX_S / SV,
                        in1=rcb[:], op0=OP.mult, op1=OP.mult)

            # ================= out-proj + meta MLP =================
            with tc.tile_pool(name="attp", bufs=1) as attp, \
                 tc.tile_pool(name="h1p_", bufs=1) as h1pp, \
                 tc.tile_pool(name="sml", bufs=1) as sml, \
                 tc.tile_pool(name="ps2", bufs=3, space="PSUM") as ps2, \
                 tc.tile_pool(name="ps3", bufs=2, space="PSUM") as ps3:
                att_b = attp.tile([P, 8, SQ], bf16, tag="att_b")
                h1p = h1pp.tile([P, 2, SQ], bf16, tag="h1p")
                h1sq = h1pp.tile([P, 2, SQ], bf16, tag="h1sq")
                h1n = h1pp.tile([P, 2, SQ], bf16, tag="h1n")
                stat = h1pp.tile([P, 2, SQ], f32, tag="stat")
                nmean, work = stat[:, 0, :], stat[:, 1, :]
                res_sb = h1pp.tile([P, 8], f32, tag="res")
                hb2_all = h1pp.tile([P, 8, MD2], f32, tag="hb2_all")

                # out-projection (fp8 DR over ctx features, full-width)
                for dt in range(8):
                    pso = ps2.tile([P, SQ], f32, tag="mmw", name="pso")
                    for hb in range(2):
                        hsl = slice(hb * 512, (hb + 1) * 512)
                        for c4 in range(4):
                            nc.tensor.matmul(
                                pso[:, hsl],
                                lhsT=ow[:, 2 * c4:2 * c4 + 2, dt * P:(dt + 1) * P],
                                rhs=ctx8[:, 2 * c4:2 * c4 + 2, hsl],
                                perf_mode=DR, start=(c4 == 0), stop=(c4 == 3))
                    nc.scalar.activation(att_b[:, dt, :], pso[:], ACT.Identity,
                                         bias=ob_sb[:, dt:dt + 1],
                                         scale=1.0 / (CTX_S * SO))
                # h1 attention part (bf16) + combine with early x part
                for ft in range(2):
                    psa = ps2.tile([P, SQ], f32, tag="mmw", name="psa")
                    for hb in range(2):
                        hsl = slice(hb * 512, (hb + 1) * 512)
                        for c8 in range(8):
                            nc.tensor.matmul(
                                psa[:, hsl],
                                lhsT=w1b[:, 8 + c8, ft * P:(ft + 1) * P],
                                rhs=att_b[:, c8, hsl],
                                start=(c8 == 0), stop=(c8 == 7))
                    nc.vector.tensor_tensor(out=h1p[:, ft, :], in0=psa[:],
                                            in1=h1x[:, ft, :], op=OP.add)
                    nc.vector.tensor_tensor(out=h1sq[:, ft, :],
                                            in0=h1p[:, ft, :],
                                            in1=h1p[:, ft, :], op=OP.mult)
                # LN1 stats via ones-matmul column sums (full width)
                psA = ps2.tile([P, SQ], f32, tag="mmw", name="psA")
                psB = ps2.tile([P, SQ], f32, tag="mmw", name="psB")
                for hb in range(2):
                    hsl = slice(hb * 512, (hb + 1) * 512)
                    for ft in range(2):
                        nc.tensor.matmul(psA[:, hsl], lhsT=onesb[:],
                                         rhs=h1p[:, ft, hsl],
                                         start=(ft == 0), stop=(ft == 1))
                    for ft in range(2):
                        nc.tensor.matmul(psB[:, hsl], lhsT=onesb[:],
                                         rhs=h1sq[:, ft, hsl],
                                         start=(ft == 0), stop=(ft == 1))
                nc.vector.tensor_scalar_mul(nmean[:], psA[:], -1.0 / MD)
                nc.vector.tensor_scalar_mul(work[:], psB[:], 1.0 / MD)
                m2 = sml.tile([P, SQ], f32, tag="m2")
                nc.vector.tensor_tensor(out=m2[:], in0=nmean[:], in1=nmean[:],
                                        op=OP.mult)
                nc.vector.tensor_tensor(out=work[:], in0=work[:], in1=m2[:],
                                        op=OP.subtract)
                # rstd = exp(-0.5 * ln(var + eps)) on ACT
                nc.scalar.activation(work[:], work[:], ACT.Ln,
                                     bias=eps_sb[:, 0:1], scale=1.0)
                nc.scalar.activation(work[:], work[:], ACT.Exp,
                                     bias=0.0, scale=-0.5)
                for ft in range(2):
                    nc.vector.tensor_tensor(out=h1n[:, ft, :],
                                            in0=h1p[:, ft, :],
                                            in1=nmean[:], op=OP.add)
                    nc.vector.tensor_tensor(out=h1n[:, ft, :],
                                            in0=h1n[:, ft, :],
                                            in1=work[:], op=OP.mult)
                    nc.scalar.activation(h1n[:, ft, :], h1n[:, ft, :],
                                         ACT.Relu, bias=be1_sb[:, ft:ft + 1],
                                         scale=g1_sb[:, ft:ft + 1])

                # h2 + LN2 + final, token-major
                F2 = float(MD2)
                for tt in range(8):
                    ph2 = ps3.tile([P, 512], f32, tag="mm512", name="ph2")
                    for ft in range(2):
                        nc.tensor.matmul(ph2[:, :MD2],
                                         lhsT=h1n[:, ft, tt * P:(tt + 1) * P],
                                         rhs=w2v[:, ft, :],
                                         start=(ft == 0), stop=(ft == 1))
                    nc.vector.tensor_tensor(out=hb2_all[:, tt, :],
                                            in0=ph2[:, :MD2], in1=b2_sb[:],
                                            op=OP.add)
                sums2 = sml.tile([P, 8], f32, tag="sums2")
                nc.vector.reduce_sum(sums2[:], hb2_all[:],
                                     axis=mybir.AxisListType.X)
                msq = sml.tile([P, 8, MD2], f32, tag="msq")
                nc.vector.tensor_tensor(out=msq[:], in0=hb2_all[:],
                                        in1=hb2_all[:], op=OP.mult)
                ssq2 = sml.tile([P, 8], f32, tag="ssq2")
                nc.vector.reduce_sum(ssq2[:], msq[:], axis=mybir.AxisListType.X)
                nm2 = sml.tile([P, 8], f32, tag="nm2")
                nc.vector.tensor_scalar_mul(nm2[:], sums2[:], -1.0 / F2)
                ex22 = sml.tile([P, 8], f32, tag="ex22")
                nc.vector.tensor_scalar_mul(ex22[:], ssq2[:], 1.0 / F2)
                mm2 = sml.tile([P, 8], f32, tag="mm2")
                nc.vector.tensor_tensor(out=mm2[:], in0=nm2[:], in1=nm2[:],
                                        op=OP.mult)
                var2 = sml.tile([P, 8], f32, tag="var2")
                nc.vector.tensor_tensor(out=var2[:], in0=ex22[:], in1=mm2[:],
                                        op=OP.subtract)
                std2 = sml.tile([P, 8], f32, tag="std2")
                nc.scalar.activation(std2[:], var2[:], ACT.Sqrt,
                                     bias=eps_sb[:, 0:1], scale=1.0)
                rstd2 = sml.tile([P, 8], f32, tag="rstd2")
                with nc.allow_low_precision(reason="ln2 recip"):
                    nc.vector.reciprocal(rstd2[:], std2[:])
                t1a = sml.tile([P, 8, MD2], f32, tag="t1a")
                nc.vector.tensor_tensor(
                    out=t1a[:], in0=hb2_all[:],
                    in1=nm2[:, :, None].to_broadcast([P, 8, MD2]), op=OP.add)
                nc.vector.tensor_tensor(
                    out=t1a[:], in0=t1a[:],
                    in1=rstd2[:, :, None].to_broadcast([P, 8, MD2]), op=OP.mult)
                nc.vector.tensor_tensor(
                    out=t1a[:], in0=t1a[:],
                    in1=g2_sb[:, None, :].to_broadcast([P, 8, MD2]), op=OP.mult)
                nc.vector.tensor_tensor(
                    out=t1a[:], in0=t1a[:],
                    in1=be2_sb[:, None, :].to_broadcast([P, 8, MD2]), op=OP.add)
                nc.vector.tensor_scalar_max(t1a[:], t1a[:], 0.0)
                nc.vector.tensor_tensor(
                    out=t1a[:], in0=t1a[:],
                    in1=w3_sb[:, None, :].to_broadcast([P, 8, MD2]), op=OP.mult)
                base8 = sml.tile([P, 8], f32, tag="base8")
                nc.vector.reduce_sum(base8[:], t1a[:], axis=mybir.AxisListType.X)
                nc.vector.tensor_tensor(
                    out=base8[:], in0=base8[:],
                    in1=b3_sb[:, 0:1].to_broadcast([P, 8]), op=OP.add)
                imp1a = sml.tile([P, 8], f32, tag="imp1a")
                nc.vector.tensor_scalar_add(imp1a[:], imp_all[:], 1.0)
                nc.vector.tensor_tensor(out=base8[:], in0=base8[:],
                                        in1=imp1a[:], op=OP.mult)
                nc.vector.tensor_scalar(base8[:], base8[:], MAX_W, MIN_W,
                                        op0=OP.min, op1=OP.max)
                nc.vector.tensor_tensor(out=res_sb[:], in0=base8[:],
                                        in1=maskf_sb[:], op=OP.mult)
                nc.sync.dma_start(out[:].rearrange("(t p) -> p t", p=P),
                                  res_sb[:])

    nc.compile()
    return nc


def _maybe_enable_ldw_opt():
    """Opt into walrus's LDWEIGHTS optimization pass (hoists/merges weight
    loads) for this kernel's own compilation when KB_LDW_OPT=1."""
    if os.environ.get("KB_LDW_OPT", "0") != "1":
        return
    import concourse.bass_utils as bu
    if getattr(bu, "_kb_ldw_patched", False):
        return
    orig = bu.run_command

    def patched(cmd, *a, **kw):
        if isinstance(cmd, list):
            cmd = ["--enable-ldw-opt=true" if c == "--enable-ldw-opt=false"
                   else c for c in cmd]
        return orig(cmd, *a, **kw)

    bu.run_command = patched
    bu._kb_ldw_patched = True


def _get_program():
    _maybe_enable_ldw_opt()
    dve_kt = int(os.environ.get("KB_DVE_KT", "0"))
    pool_kt = int(os.environ.get("KB_POOL_KT", "0"))
    key = ("nc", dve_kt, pool_kt)
    if key not in _CACHE:
        _CACHE[key] = _build(dve_kt, pool_kt)
    return _CACHE[key]


def _pack8(mat):
    """[R, C] with R = nchunk*128 -> [128, nchunk*C] chunk-major layout."""
    r, c = mat.shape
    nchunk = r // P
    return np.ascontiguousarray(
        mat.reshape(nchunk, P, c).transpose(1, 0, 2).reshape(P, nchunk * c))


def _prep_in_maps(inputs):
    import ml_dtypes
    f8 = ml_dtypes.float8_e4m3
    bf16 = ml_dtypes.bfloat16

    hidden = np.asarray(inputs["hidden_states"], dtype=np.float32)
    token_ids = np.asarray(inputs["token_ids"], dtype=np.int32)
    mask = np.asarray(inputs["attention_mask"]).astype(bool)
    pos = np.asarray(inputs["pos_embed"], dtype=np.float32)
    in_proj_w = np.asarray(inputs["in_proj_w"], dtype=np.float32)
    in_proj_b = np.asarray(inputs["in_proj_b"], dtype=np.float32)
    out_w = np.asarray(inputs["out_w"], dtype=np.float32)
    out_b = np.asarray(inputs["out_b"], dtype=np.float32)
    w1 = np.asarray(inputs["w1"], dtype=np.float32)
    b1 = np.asarray(inputs["b1"], dtype=np.float32)
    g1 = np.asarray(inputs["g1"], dtype=np.float32)
    beta1 = np.asarray(inputs["beta1"], dtype=np.float32)
    w2 = np.asarray(inputs["w2"], dtype=np.float32)
    b2 = np.asarray(inputs["b2"], dtype=np.float32)
    g2 = np.asarray(inputs["g2"], dtype=np.float32)
    beta2 = np.asarray(inputs["beta2"], dtype=np.float32)
    w3 = np.asarray(inputs["w3"], dtype=np.float32)
    b3 = np.asarray(inputs["b3"], dtype=np.float32)
    table = np.asarray(inputs["importance_table"], dtype=np.float32)

    B, S_, H_ = hidden.shape
    assert (B, S_, H_) == (4, S, H), (B, S_, H_)

    x = hidden + pos[:, :S, :]                             # [B, S, H]

    def cmaj(v):   # [nchunk*128] -> [128, nchunk] partition-major
        return np.ascontiguousarray(v.reshape(-1, P).T)

    def bcast(v):  # [F] -> [128, F]
        return np.ascontiguousarray(np.broadcast_to(v[None, :], (P, v.shape[0])))

    wq8 = _pack8((in_proj_w[0:H] * SW).T).astype(f8)
    wk8 = _pack8((in_proj_w[H:2 * H] * SW).T).astype(f8)
    wv8 = _pack8((in_proj_w[2 * H:3 * H] * SV).T).astype(f8)
    ow8 = _pack8((out_w * SO).T).astype(f8)
    w1b = _pack8(w1.T).astype(bf16)                        # [2H, MD] packed
    w2b = _pack8(w2.T).astype(bf16)                        # [MD, MD2] packed

    shared = {
        "wq8": wq8, "wk8": wk8, "wv8": wv8, "ow8": ow8,
        "w1bd": w1b, "w2bd": w2b,
        "bq_c": cmaj(in_proj_b[0:H] * SW).astype(np.float32),
        "bk_c": cmaj(in_proj_b[H:2 * H] * SW).astype(np.float32),
        "bv_b": bcast(in_proj_b[2 * H:3 * H] * SV).astype(np.float32),
        "ob_c": cmaj(out_b).astype(np.float32),
        "b1_c": cmaj(b1), "g1_c": cmaj(g1), "be1_c": cmaj(beta1),
        "b2_b": bcast(b2), "g2_b": bcast(g2), "be2_b": bcast(beta2),
        "w3_b": bcast(w3[0]), "b3_c": np.full((P, 1), b3[0], dtype=np.float32),
        "table": np.ascontiguousarray(table[:, None]),
    }

    in_maps = []
    for c in range(8):
        b = c // 2
        half = c % 2
        own = slice(half * SQ, (half + 1) * SQ)
        oth = slice((1 - half) * SQ, (2 - half) * SQ)
        xT = x[b].T                                        # [H, S]
        xT_arr = np.concatenate([xT[:, own], xT[:, oth]], axis=1)
        mb = np.where(mask[b], 0.0, -1e9).astype(np.float32)
        mb_arr = np.concatenate([mb[own], mb[oth]])        # key-order remap
        kb = (mb_arr - EXP_SHIFT).reshape(16, P).T         # [128, 16]
        dB = (mb_arr * A8 + (56.0 - EXP_SHIFT * A8)).reshape(16, P).T
        m = {
            "x8d": _pack8(xT_arr).astype(f8),
            "xbd": _pack8(np.ascontiguousarray(xT_arr[:, :SQ])).astype(bf16),
            "kbias": np.ascontiguousarray(kb),
            "dveB": np.ascontiguousarray(dB),
            "maskf": np.ascontiguousarray(
                mask[b, own].astype(np.float32).reshape(-1, P).T),
            "tok": np.ascontiguousarray(token_ids[b, own][:, None]),
        }
        m.update(shared)
        in_maps.append(m)
    return in_maps


def _assemble(res):
    full = np.zeros((4, S), dtype=np.float32)
    for c in range(8):
        b = c // 2
        half = c % 2
        full[b, half * SQ:(half + 1) * SQ] = res.results[c]["out"]
    return full


def kernel(**inputs) -> np.ndarray:
    from concourse.bass_utils import run_bass_kernel_spmd
    in_maps = _prep_in_maps(inputs)
    nc = _get_program()
    res = run_bass_kernel_spmd(nc, in_maps, list(range(8)))
    return _assemble(res)


def run_traced(inputs, **kwargs):
    from concourse.bass_utils import run_bass_kernel_spmd
    in_maps = _prep_in_maps(inputs)
    nc = _get_program()
    return run_bass_kernel_spmd(nc, in_maps, list(range(8)), trace=True, **kwargs)
